# revision 1
# baseline (speedup 1.0000x reference)
"""Trainium2 Bass kernel for FlowNet-C CorrelationCost.

Problem: out[b,i,j, tj*21+ti] = (1/C) * sum_c A[b,i,j,c] * Bz[b, i+dy, j+dx, c]
with dy = 2*tj - 20, dx = 2*ti - 20, Bz = B zero-padded by 20 spatially.
Shapes: A, B = [16, 48, 64, 256] f32 -> out [16, 48, 64, 441] f32.

Strategy
--------
- Pure data-parallel: batch 16 -> 2 images per NeuronCore (8 cores, SPMD).
- Host pre-transposes inputs to channel-major [b, C, H, W] so DMA loads land
  in SBUF as [C-chunk(128) partitions, spatial] directly (no on-chip transpose).
- PE formulation: contract over C. For an i-pack {i0, i0+2, i0+4, i0+6} (same
  parity) and a column-parity class p, the stationary operand is
  A[c, pack x 32 same-parity columns] (128x128) and the moving operand streams
  B[c, r x 32 same-parity columns] for all B rows r with |r - i| <= 20 for some
  i in the pack. PSUM[m=(i,j), n=(r,jj)] then holds every correlation product
  with dy = r - i, dx = jj - j (dx even => j, jj same parity => parity split).
- fp32 is split on the host into fp16 hi+lo (prescaled by 1/16 each so the
  1/C output scale is baked in); hi*hi + hi*lo + lo*hi accumulate as six
  128-channel K-passes in fp32 PSUM (~6e-7 rel err at full PE rate).
- VectorE copies PSUM -> SBUF, one ACT-ring DMA per chunk ships the whole
  [128, ncols] block to DRAM. The host extracts the valid diagonal band
  (numpy as_strided) and assembles the output. The first 20 chunks run
  two-phase (hi*hi early, cross-terms added once the lo data lands) so the
  PE ramps while inputs stream in.

The harness calls kernel(**inputs) with the FULL inputs; this file is
self-contained (shapes hardcoded).
"""

import os
from contextlib import ExitStack

import numpy as np

import concourse.bass as bass
import concourse.tile as tile
from concourse import bacc, mybir

B_FULL, H, W, C = 16, 48, 64, 256
N_CORES = 8
B_PER = B_FULL // N_CORES  # batches per core
MD = 20                    # max displacement
D = 21                     # displacements per axis
PACK = 4                   # i rows packed into one stationary operand
NCOLS_MAX = 512            # one fp32 PSUM bank
F32 = mybir.dt.float32
F32R = mybir.dt.float32r   # fp32 PE fast path
BF16 = mybir.dt.bfloat16

# "fp16hl": split fp32 into fp16 hi+lo; compute hi*hi + hi*lo + lo*hi as 6
#           accumulating K-passes (PSUM is fp32). ~1e-7 rel err, full-rate PE
#           and fast (FWL) weight loads, same DMA bytes as fp32. Default.
# "f32r":   fp32 fast-ish PE path, ~1.5e-4 rel err.  "f32": exact, 4x slower.
MODE = os.environ.get("CORR_MODE", "fp16hl")
HL = MODE == "fp16hl"
C_K = 512 if HL else 256   # device-side channels (hi+lo stacked)
# (a-chunk, b-chunk) 128-channel pass pairs accumulated into PSUM:
# chunks 0,1 = hi, 2,3 = lo;  hi*hi + hi*lo + lo*hi.
HL_PASSES = [(0, 0), (1, 1), (2, 0), (0, 2), (3, 1), (1, 3)]
TWO_PHASE = 20   # b0 chunks whose hi*hi chain runs before the lo data lands


def plan_groups():
    """(pack, r_list) per i-pack: pack = 4 same-parity rows, r_list = B rows
    (same parity, step 2) needed by any row in the pack."""
    groups = []
    for par in (0, 1):
        i_vals = list(range(par, H, 2))
        for k in range(0, len(i_vals), PACK):
            pack = i_vals[k:k + PACK]
            r_lo = max(0, pack[0] - MD)
            r_hi = min(H - 1, pack[-1] + MD)
            r_list = [r for r in range(r_lo, r_hi + 1) if (r - pack[0]) % 2 == 0]
            groups.append((pack, r_list))
    return groups


def chunk_rs(r_list):
    """Split the r list into chunks of <= 16 rows (<= 512 cols, one PSUM bank),
    keeping every chunk >= 8 rows (256 cols) for the f32r full-rate path."""
    n = len(r_list)
    if n <= 16:
        return [r_list]
    h = (n + 1) // 2
    return [r_list[:h], r_list[h:]]


GROUPS = plan_groups()
N_GROUPS = len(GROUPS)            # 12 i-packs
MAX_CHUNKS = max(len(chunk_rs(r)) for _, r in GROUPS)  # 2


def pack_inputs(a_t, b_t):
    """Channel-major [b, C, H, W] -> matmul-ready packed layouts.

    a_packed[b, c, par, pk, p, k, j32] = a_t[b, c, 8*pk + 2*k + par, 2*j32 + p]
    b_packed[b, c, p, par, r2, jj32]  = b_t[b, c, 2*r2 + par, 2*jj32 + p]

    so that lhsT = a[:, par, pk, p, :] and rhs = b[:, p, par, r2 slice, :] are
    single-free-dim contiguous APs (a BIR matmul requirement).
    """
    nb, ck = a_t.shape[0], a_t.shape[1]
    ap = a_t.reshape(nb, ck, 6, PACK, 2, 32, 2).transpose(0, 1, 4, 2, 6, 3, 5)
    bp = b_t.reshape(nb, ck, 24, 2, 32, 2).transpose(0, 1, 5, 3, 2, 4)
    return (np.ascontiguousarray(ap).reshape(nb, ck, 2, 6, 2, PACK * 32),
            np.ascontiguousarray(bp).reshape(nb, ck, 2, 2, 24 * 32))


def to_device_channels(x_t):
    """[nb, C, H, W] f32 -> [nb, C_K, H, W] in the device dtype.

    fp16hl: channels 0..C-1 = fp16 round of x/16, C..2C-1 = fp16 round of
    the residual (x/16 = hi + lo to ~2^-24). The 1/16 per input bakes the
    1/C = 1/256 output scale into the product (exact: power of two)."""
    if not HL:
        return x_t
    y = x_t * np.float32(1.0 / 16.0)
    hi = y.astype(np.float16)
    lo = (y - hi.astype(np.float32)).astype(np.float16)
    return np.concatenate([hi, lo], axis=1)


def build_program():
    nc = bacc.Bacc("TRN2", target_bir_lowering=False, debug=False)

    mm_dt = {"fp16hl": mybir.dt.float16, "f32r": F32R, "f32": F32}[MODE]
    a_d = nc.dram_tensor("a_t", [B_PER, C_K, 2, 6, 2, PACK * 32], mm_dt,
                         kind="ExternalInput")
    b_d = nc.dram_tensor("b_t", [B_PER, C_K, 2, 2, 24 * 32], mm_dt,
                         kind="ExternalInput")
    # raw matmul blocks: [b, group, parity, chunk, 128, 512]
    o_d = nc.dram_tensor(
        "out_raw", [B_PER, N_GROUPS, 2, MAX_CHUNKS, 128, NCOLS_MAX], F32,
        kind="ExternalOutput",
    )

    with tile.TileContext(nc) as tc, ExitStack() as ctx:
        inp = ctx.enter_context(tc.tile_pool(name="inp", bufs=1))
        psum = ctx.enter_context(
            tc.tile_pool(name="psum", bufs=8, space=bass.MemorySpace.PSUM))
        stage = ctx.enter_context(tc.tile_pool(name="stage", bufs=12))
        histage = ctx.enter_context(tc.tile_pool(name="histage", bufs=1))

        # Input loads. b0's hi chunks arrive as fine-grained quarters split
        # across BOTH HWDGE rings so the first matmuls unblock ~1.5us in;
        # everything later arrives as big per-(b,cc) tiles (fewer DMAs, full
        # ring throughput) on the SP ring while the PE is already busy.
        # Output stores go on the ACT ring (idle once the bootstrap quarters
        # finish); PSUM copies all run on VectorE. Every engine queue is
        # feed-forward: no instruction ever waits behind an unrelated one.
        a_sb = {}
        b_sb = {}

        def load_big(eng, b, cc):
            cs = slice(cc * 128, (cc + 1) * 128)
            ta = inp.tile([128, 2, 6, 2, PACK * 32], mm_dt, tag=f"ab{b}_{cc}")
            eng.dma_start(ta[:], a_d[b, cs])
            a_sb[b, cc] = ta
            tb = inp.tile([128, 2, 2, 24 * 32], mm_dt, tag=f"bb{b}_{cc}")
            eng.dma_start(tb[:], b_d[b, cs])
            b_sb[b, cc] = tb

        # Bootstrap: b0 hi chunks as fine quarters, A on the SP ring, B on
        # the ACT ring, so the first matmul unblocks ~1.5us after the
        # preamble. The rest arrives as big per-(b,cc) tiles on the SP ring.
        for cc in (0, 1):
            cs = slice(cc * 128, (cc + 1) * 128)
            for par in (0, 1):
                ta = inp.tile([128, 6, 2, PACK * 32], mm_dt,
                              tag=f"aq{cc}_{par}")
                nc.sync.dma_start(ta[:], a_d[0, cs, par])
                a_sb[0, cc, par] = ta
                for p in (0, 1):
                    tb = inp.tile([128, 24 * 32], mm_dt, tag=f"bq{cc}_{p}{par}")
                    nc.scalar.dma_start(tb[:], b_d[0, cs, p, par])
                    b_sb[0, cc, p, par] = tb
        load_big(nc.sync, 0, 2)
        load_big(nc.sync, 0, 3)
        for cc in range(C_K // 128):
            load_big(nc.sync, 1, cc)

        def lhs_ap(b, cc, par, pk, p):
            if (b, cc, par) in a_sb:
                return a_sb[b, cc, par][:, pk, p, :]
            return a_sb[b, cc][:, par, pk, p, :]

        def rhs_ap(b, cc, p, par, lo, hi):
            if (b, cc, p, par) in b_sb:
                return b_sb[b, cc, p, par][:, lo:hi]
            return b_sb[b, cc][:, p, par, lo:hi]

        # Chunk worklist in processing order.
        work = []
        for b in range(B_PER):
            for gi, (pack, r_list) in enumerate(GROUPS):
                par = pack[0] % 2
                pk = (pack[0] // 2) // PACK
                for p in (0, 1):
                    for ci, rs in enumerate(chunk_rs(r_list)):
                        work.append((b, gi, par, pk, p, ci, rs))

        hi_passes = HL_PASSES[:2]
        lo_passes = HL_PASSES[2:]
        all_passes = HL_PASSES if HL else [(cc, cc) for cc in range(C_K // 128)]

        def run_passes(ps, passes, b, par, pk, p, r2lo, nr, ncols):
            for pi, (ca, cb) in enumerate(passes):
                rhs = rhs_ap(b, cb, p, par, r2lo * 32, (r2lo + nr) * 32)
                nc.tensor.matmul(
                    ps[:, :ncols], lhs_ap(b, ca, par, pk, p), rhs,
                    start=(pi == 0), stop=(pi == len(passes) - 1),
                )

        # Phase 1: for the first TP chunks (b0), run just the hi*hi chains —
        # they only need the bootstrap quarters, so they close (and free
        # their PSUM bank) long before the lo chunks arrive. Results are
        # staged in SBUF; the cross-product chains add onto them later.
        # Inputs are host-prescaled by 1/16 each, so no 1/C scale is needed.
        TP = TWO_PHASE if HL else 0
        hi_st = {}
        for w in work[:TP]:
            b, gi, par, pk, p, ci, rs = w
            r2lo, nr = rs[0] // 2, len(rs)
            ncols = nr * 32
            ps = psum.tile([128, NCOLS_MAX], F32, tag="ps")
            run_passes(ps, hi_passes, b, par, pk, p, r2lo, nr, ncols)
            hs = histage.tile([128, NCOLS_MAX], F32, tag=f"hs{len(hi_st)}")
            nc.vector.tensor_copy(hs[:, :ncols], ps[:, :ncols])
            hi_st[w[:6]] = hs

        # Phase 2 / single-phase processing of every chunk in order.
        for wi, w in enumerate(work):
            b, gi, par, pk, p, ci, rs = w
            r2lo, nr = rs[0] // 2, len(rs)
            ncols = nr * 32
            ps = psum.tile([128, NCOLS_MAX], F32, tag="ps")
            st = stage.tile([128, NCOLS_MAX], F32, tag="st")
            if wi < TP:
                run_passes(ps, lo_passes, b, par, pk, p, r2lo, nr, ncols)
                nc.vector.tensor_add(
                    st[:, :ncols], ps[:, :ncols], hi_st[w[:6]][:, :ncols])
            else:
                run_passes(ps, all_passes, b, par, pk, p, r2lo, nr, ncols)
                nc.vector.tensor_copy(st[:, :ncols], ps[:, :ncols])
            nc.scalar.dma_start(o_d[b, gi, p, ci, :, :ncols], st[:, :ncols])

    nc.compile()
    return nc


_NC_CACHE = None


def _get_program():
    global _NC_CACHE
    if _NC_CACHE is None:
        _NC_CACHE = build_program()
    return _NC_CACHE


def assemble_output(raw_all):
    """raw_all: [nb, N_GROUPS, 2, MAX_CHUNKS, 128, 512] f32 (already scaled)
    -> out [nb, H, W, D*D] f32."""
    nb = raw_all.shape[0]
    # band tensor: [nb, H, 2(p), 32(j32), D(dy), 32(jj32)]
    band = np.zeros((nb, H, 2, 32, D, 32), np.float32)
    for gi, (pack, r_list) in enumerate(GROUPS):
        for ci, rs in enumerate(chunk_rs(r_list)):
            nr = len(rs)
            # [B, 2p, 128, nr*32] -> [B, 2p, 4i, 32j, nr, 32jj]
            blk = raw_all[:, gi, :, ci, :, :nr * 32].reshape(
                nb, 2, PACK, 32, nr, 32)
            for k, i in enumerate(pack):
                for ridx, r in enumerate(rs):
                    dy = r - i
                    if abs(dy) > MD:
                        continue
                    dyi = (dy + MD) // 2
                    # [B, 2p, 32j, 32jj] -> band[:, i, p, j32, dyi, jj32]
                    band[:, i, :, :, dyi, :] = blk[:, :, k, :, ridx, :]
    out = np.zeros((nb, H, W, D, D), np.float32)
    s = band.strides
    for p in (0, 1):
        for ti in range(D):
            delta = ti - MD // 2  # dx/2
            j32_lo = max(0, -delta)
            j32_hi = min(32, 32 - delta)
            n = j32_hi - j32_lo
            if n <= 0:
                continue
            v = np.lib.stride_tricks.as_strided(
                band[:, :, p, j32_lo:, :, j32_lo + delta:],
                shape=(nb, H, n, D),
                strides=(s[0], s[1], s[3] + s[5], s[4]),
            )
            out[:, :, 2 * np.arange(j32_lo, j32_hi) + p, :, ti] = \
                v.transpose(2, 0, 1, 3)
    return out.reshape(nb, H, W, D * D)


def kernel(input_a: np.ndarray, input_b: np.ndarray) -> np.ndarray:
    from concourse.bass_utils import run_bass_kernel_spmd

    a = np.asarray(input_a, np.float32).transpose(0, 3, 1, 2)  # [B, C, H, W]
    bt = np.asarray(input_b, np.float32).transpose(0, 3, 1, 2)
    a, bt = pack_inputs(to_device_channels(a), to_device_channels(bt))

    nc = _get_program()
    core_ids = list(range(N_CORES))
    in_maps = [
        {"a_t": a[c * B_PER:(c + 1) * B_PER], "b_t": bt[c * B_PER:(c + 1) * B_PER]}
        for c in core_ids
    ]
    res = run_bass_kernel_spmd(nc, in_maps, core_ids)
    raw_all = np.concatenate(
        [res.results[c]["out_raw"] for c in core_ids], axis=0)
    return assemble_output(raw_all)



# revision 2
# speedup vs baseline: 1.7939x; 1.7939x over previous
"""Trainium2 Bass kernel for FlowNet-C CorrelationCost.

Problem: out[b,i,j, tj*21+ti] = (1/C) * sum_c A[b,i,j,c] * Bz[b, i+dy, j+dx, c]
with dy = 2*tj - 20, dx = 2*ti - 20, Bz = B zero-padded by 20 spatially.
Shapes: A, B = [16, 48, 64, 256] f32 -> out [16, 48, 64, 441] f32.

Strategy (v2)
-------------
- Pure data-parallel: batch 16 -> 2 images per NeuronCore (8 cores, SPMD).
- Host pre-transposes to channel-major, prescales by 1/16 (so the two input
  scales bake the exact 1/256 output scale) and rounds to fp16. Tolerance is
  2e-2; fp16 inputs give ~1.5e-4, so no hi/lo split -> only 2 K-passes of
  128 channels each (3x less PE work than the fp32-exact hi/lo scheme).
- PE formulation: stationary = A[c, 4 same-parity rows x 32 same-parity
  cols] (128x128); moving = B[c, r x 32 same-parity cols] for all B rows r
  within +-20 of the pack. PSUM[(i,j), (r,jj)] holds every correlation
  product with dy = r - i, dx = jj - j.
- Per (batch, group, col-parity): one 2-bank PSUM supertile holds both
  r-chunks; the 2 matmul passes accumulate fp32; ONE copy (alternating
  VectorE / ScalarE) casts PSUM -> fp16 SBUF; ONE DMA (alternating SP ring /
  Pool ring) ships the compact block. Output bytes are halved vs fp32.
- Input DMAs: batch 0 arrives as fine-grained quarters on the SP ring (PE
  unblocks ~2.5us in); batch 1 as big per-(b,chunk) tiles on the Pool ring.

The harness calls kernel(**inputs) with the FULL inputs; this file is
self-contained (shapes hardcoded).
"""

from contextlib import ExitStack

import numpy as np

import concourse.bass as bass
import concourse.tile as tile
from concourse import bacc, mybir

B_FULL, H, W, C = 16, 48, 64, 256
N_CORES = 8
B_PER = B_FULL // N_CORES  # batches per core
MD = 20                    # max displacement
D = 21                     # displacements per axis
PACK = 4                   # i rows packed into one stationary operand
F32 = mybir.dt.float32
F16 = mybir.dt.float16
N_CC = C // 128            # channel chunks (K-passes)
OUT_W = 704                # max per-(b,gi,p) out cols (2 chunks x 11 r x 32)


def plan_groups():
    """(pack, r_list) per i-pack: pack = 4 same-parity rows, r_list = B rows
    (same parity, step 2) needed by any row in the pack."""
    groups = []
    for par in (0, 1):
        i_vals = list(range(par, H, 2))
        for k in range(0, len(i_vals), PACK):
            pack = i_vals[k:k + PACK]
            r_lo = max(0, pack[0] - MD)
            r_hi = min(H - 1, pack[-1] + MD)
            r_list = [r for r in range(r_lo, r_hi + 1) if (r - pack[0]) % 2 == 0]
            groups.append((pack, r_list))
    return groups


def chunk_rs(r_list):
    """Split the r list into chunks of <= 16 rows (<= 512 cols, one PSUM
    bank)."""
    n = len(r_list)
    if n <= 16:
        return [r_list]
    h = (n + 1) // 2
    return [r_list[:h], r_list[h:]]


GROUPS = plan_groups()
N_GROUPS = len(GROUPS)  # 12 i-packs


def prep_inputs(input_a, input_b):
    """Full [B, H, W, C] f32 inputs -> packed fp16 device arrays.

    a_packed[b, c, par, pk, p, k*32+j32] = a[b, 8*pk+2*k+par, 2*j32+p, c]/16
    b_packed[b, c, p, par, r2*32+jj32]  = b[b, 2*r2+par, 2*jj32+p, c]/16
    """
    s = np.float16(1.0)  # applied after f32 * (1/16)
    at = (np.asarray(input_a, np.float32) * np.float32(1 / 16)).astype(np.float16)
    bt = (np.asarray(input_b, np.float32) * np.float32(1 / 16)).astype(np.float16)
    at = at.transpose(0, 3, 1, 2)  # [B, C, H, W]
    bt = bt.transpose(0, 3, 1, 2)
    nb = at.shape[0]
    ap = at.reshape(nb, C, 6, PACK, 2, 32, 2).transpose(0, 1, 4, 2, 6, 3, 5)
    bp = bt.reshape(nb, C, 24, 2, 32, 2).transpose(0, 1, 5, 3, 2, 4)
    del s
    return (np.ascontiguousarray(ap).reshape(nb, C, 2, 6, 2, PACK * 32),
            np.ascontiguousarray(bp).reshape(nb, C, 2, 2, 24 * 32))


def build_program():
    nc = bacc.Bacc("TRN2", target_bir_lowering=False, debug=False)

    a_d = nc.dram_tensor("a_t", [B_PER, C, 2, 6, 2, PACK * 32], F16,
                         kind="ExternalInput")
    b_d = nc.dram_tensor("b_t", [B_PER, C, 2, 2, 24 * 32], F16,
                         kind="ExternalInput")
    o_d = nc.dram_tensor("out_raw", [B_PER, N_GROUPS, 2, 128, OUT_W], F16,
                         kind="ExternalOutput")

    with tile.TileContext(nc) as tc, ExitStack() as ctx:
        inp = ctx.enter_context(tc.tile_pool(name="inp", bufs=1))
        psum = ctx.enter_context(
            tc.tile_pool(name="psum", bufs=4, space=bass.MemorySpace.PSUM))
        stage = ctx.enter_context(tc.tile_pool(name="stage", bufs=8))

        a_sb = {}
        b_sb = {}

        # Bootstrap: b0 as fine-grained quarters on the SP ring, ordered so
        # the PE's first group (par 0) unblocks after ~2 quarters.
        for par in (0, 1):
            for cc in (0, 1):
                cs = slice(cc * 128, (cc + 1) * 128)
                ta = inp.tile([128, 6, 2, PACK * 32], F16, tag=f"aq{cc}_{par}")
                nc.sync.dma_start(ta[:], a_d[0, cs, par])
                a_sb[0, cc, par] = ta
                for p in (0, 1):
                    tb = inp.tile([128, 24 * 32], F16, tag=f"bq{cc}_{p}{par}")
                    nc.sync.dma_start(tb[:], b_d[0, cs, p, par])
                    b_sb[0, cc, p, par] = tb
        # b1: big per-(chunk) tiles on the Pool ring.
        for cc in (0, 1):
            cs = slice(cc * 128, (cc + 1) * 128)
            ta = inp.tile([128, 2, 6, 2, PACK * 32], F16, tag=f"ab1_{cc}")
            nc.gpsimd.dma_start(ta[:], a_d[1, cs])
            a_sb[1, cc] = ta
            tb = inp.tile([128, 2, 2, 24 * 32], F16, tag=f"bb1_{cc}")
            nc.gpsimd.dma_start(tb[:], b_d[1, cs])
            b_sb[1, cc] = tb

        def lhs_ap(b, cc, par, pk, p):
            if (b, cc, par) in a_sb:
                return a_sb[b, cc, par][:, pk, p, :]
            return a_sb[b, cc][:, par, pk, p, :]

        def rhs_ap(b, cc, p, par, lo, hi):
            if (b, cc, p, par) in b_sb:
                return b_sb[b, cc, p, par][:, lo:hi]
            return b_sb[b, cc][:, p, par, lo:hi]

        t = 0
        for b in range(B_PER):
            for gi, (pack, r_list) in enumerate(GROUPS):
                par = pack[0] % 2
                pk = (pack[0] // 2) // PACK
                chunks = chunk_rs(r_list)
                nr = len(chunks[0])
                ncols = nr * 32
                tot = len(chunks) * ncols
                for p in (0, 1):
                    ps = psum.tile([128, 1024], F32, tag="ps")
                    for cc in (0, 1):
                        for ci, rs in enumerate(chunks):
                            r2lo = rs[0] // 2
                            rhs = rhs_ap(b, cc, p, par,
                                         r2lo * 32, (r2lo + nr) * 32)
                            nc.tensor.matmul(
                                ps[:, ci * 512: ci * 512 + ncols],
                                lhs_ap(b, cc, par, pk, p), rhs,
                                start=(cc == 0), stop=(cc == 1),
                            )
                    st = stage.tile([128, OUT_W], F16, tag="st")
                    ps3 = ps.rearrange("q (c n) -> q c n", c=2)
                    st3 = st[:, :tot].rearrange("q (c n) -> q c n",
                                                c=len(chunks))
                    if t % 2 == 0:
                        nc.vector.tensor_copy(st3, ps3[:, :len(chunks), :ncols])
                    else:
                        nc.scalar.copy(st3, ps3[:, :len(chunks), :ncols])
                    eng = nc.sync if t % 2 == 0 else nc.gpsimd
                    eng.dma_start(o_d[b, gi, p][:, :tot], st[:, :tot])
                    t += 1

    nc.compile()
    return nc


_NC_CACHE = None


def _get_program():
    global _NC_CACHE
    if _NC_CACHE is None:
        _NC_CACHE = build_program()
    return _NC_CACHE


def assemble_output(raw_all):
    """raw_all: [nb, N_GROUPS, 2, 128, OUT_W] fp16 (scale already baked)
    -> out [nb, H, W, D*D] f32."""
    nb = raw_all.shape[0]
    raw_all = np.asarray(raw_all, np.float32)
    # band tensor: [nb, H, 2(p), 32(j32), D(dy), 32(jj32)]
    band = np.zeros((nb, H, 2, 32, D, 32), np.float32)
    for gi, (pack, r_list) in enumerate(GROUPS):
        chunks = chunk_rs(r_list)
        nr = len(chunks[0])
        for ci, rs in enumerate(chunks):
            # [nb, 2p, 128, nr*32] -> [nb, 2p, 4k, 32j, nr, 32jj]
            blk = raw_all[:, gi, :, :, ci * nr * 32:(ci + 1) * nr * 32]
            blk = blk.reshape(nb, 2, PACK, 32, nr, 32)
            for k, i in enumerate(pack):
                for ridx, r in enumerate(rs):
                    dy = r - i
                    if abs(dy) > MD:
                        continue
                    dyi = (dy + MD) // 2
                    band[:, i, :, :, dyi, :] = blk[:, :, k, :, ridx, :]
    out = np.zeros((nb, H, W, D, D), np.float32)
    s = band.strides
    for p in (0, 1):
        for ti in range(D):
            delta = ti - MD // 2  # dx/2
            j32_lo = max(0, -delta)
            j32_hi = min(32, 32 - delta)
            n = j32_hi - j32_lo
            if n <= 0:
                continue
            v = np.lib.stride_tricks.as_strided(
                band[:, :, p, j32_lo:, :, j32_lo + delta:],
                shape=(nb, H, n, D),
                strides=(s[0], s[1], s[3] + s[5], s[4]),
            )
            out[:, :, 2 * np.arange(j32_lo, j32_hi) + p, :, ti] = \
                v.transpose(2, 0, 1, 3)
    return out.reshape(nb, H, W, D * D)


def kernel(input_a: np.ndarray, input_b: np.ndarray) -> np.ndarray:
    from concourse.bass_utils import run_bass_kernel_spmd

    a, bt = prep_inputs(input_a, input_b)
    nc = _get_program()
    core_ids = list(range(N_CORES))
    in_maps = [
        {"a_t": a[c * B_PER:(c + 1) * B_PER], "b_t": bt[c * B_PER:(c + 1) * B_PER]}
        for c in core_ids
    ]
    res = run_bass_kernel_spmd(nc, in_maps, core_ids)
    raw_all = np.concatenate(
        [res.results[c]["out_raw"] for c in core_ids], axis=0)
    return assemble_output(raw_all)
